# revision 66
# baseline (speedup 1.0000x reference)
"""Bass/Tile kernel for nn_Causal_Temporal_Map_Attention_2 on 8 TRN2 NeuronCores.

Math: the reference is bilinear attention WITHOUT softmax:
    xe  = concat([x_b, e], -1)                    # (n, 512) per batch
    out = (xe Wq^T) (xe Wk^T)^T x_b * SCALE       # (n, 256)

By associativity this collapses to
    G   = xe^T x_b                                # (512, 256)   O(n d^2)
    M   = (SCALE * Wq^T Wk) G = H G               # (512, 256)
    out = xe M                                    # (n, 256)

Sharding is data-parallel over batch: core i handles batch element i
(b == n_cores == 8).

Device-side schedule (all matmuls bf16 with f32 PSUM accumulation):
  * warmup junk matmuls burn the PE p-state half-speed ramp during the DMA
    spin-up window.
  * one HWDGE input stream on the SP ring with x-half and e-half chunk
    groups INTERLEAVED, so the G phase's pass1 (x^T x rows) and pass2
    (e^T x rows) interleave per chunk group and G closes right behind the
    wire instead of serializing pass2 after the whole x stream.
  * G pass1 exploits the symmetry of the x^T x block: its (1,0) 128x128
    tile is a PE transpose of the (0,1) tile instead of 16 more matmuls.
  * out phase: PSUM chunk groups drain f32->bf16 into per-store staging
    tiles on alternating DVE/Act engines; several drain groups share one
    store DMA to keep the HWDGE slot count down.  The final group is a
    single chunk whose drain is split across both engines, so the kernel
    tail carries only half a drain + one store.
  * the kernel-end drain's semaphore waits are reordered so the final
    store's DMA lane is waited last: the one-wait-per-instruction NoOp
    chain then retires while that store's 900ns sem propagation is still
    in flight instead of after it.
"""

import os
import sys

if "/opt/trn_rl_repo" not in sys.path:
    sys.path.insert(0, "/opt/trn_rl_repo")

import numpy as np

B = 8
N = 2048
T = 256  # DIM_X
D = 512  # DIM_X + DIM_E
P = 128
NCH = N // P  # 16 sequence chunks
DCH = D // P  # 4 feature chunks
SCALE = float(D) ** -0.5

_CACHE = {}


def _split_excess_waits(nc, max_waits=1):
    """The walrus build in this container rejects instructions carrying more
    than one embedded semaphore wait ("Too many sync wait commands").  Tile's
    add_semaphores freely attaches 3+ (and the kernel-tail drain collects one
    per outstanding sem).  Rehome the excess onto nofuse NOPs prepended on the
    same engine -- the sequencer executes them in order, so blocking semantics
    are identical."""
    import concourse.mybir as mybir

    n_split = 0
    for f in nc.m.functions:
        for bb in f.blocks:
            new_insts = []
            for inst in bb.instructions:
                si = inst.sync_info
                waits = list(si.on_wait) if si is not None else []
                if len(waits) > max_waits:
                    excess = waits[: -max_waits]
                    keep = waits[-max_waits:]
                    for k in range(0, len(excess), max_waits):
                        chunk = excess[k : k + max_waits]
                        nop = mybir.InstNoOp(
                            name=f"{inst.name}-wsplit{k}",
                            engine=inst.engine,
                            ins=[],
                            outs=[],
                            text_hint="waitsplit",
                            bass_nofuse=True,
                            sync_info=mybir.SyncInfo(on_wait=chunk, on_update=[]),
                        )
                        new_insts.append(nop)
                        n_split += 1
                    inst.sync_info = mybir.SyncInfo(
                        on_wait=keep, on_update=list(si.on_update)
                    )
                new_insts.append(inst)
            bb.instructions = new_insts
    return n_split


def _patch_tail_barrier():
    """The stock kernel epilogue is drain -> all-engine barrier -> sem clear
    -> all-engine barrier.  The second barrier only keeps already-drained
    engines from halting before the sem clears land, which is harmless: NEFF
    completion requires every engine to halt, and the clearing engine halts
    after its clears.  Eliding it saves ~0.9us of tail.

    Additionally the drain's waits are reordered so the DMA-lane sems
    (DMAHW*) come last, the lane belonging to the final store very last:
    _split_excess_waits turns each wait into its own 50ns NoOp, and this
    ordering lets the early (engine) NoOps retire while the final store's
    DMA-sem propagation is still in flight."""
    import concourse.tile as tile

    if getattr(tile.TileContext, "_tail_single_barrier", False):
        return

    def _drain_and_barrier(self, tick_clock, wait_clock):
        import concourse.mybir as mybir

        nc = self.nc
        drain_inst = nc.sync.drain()
        wait_clock.add_sem_waits(
            drain_inst.ins,
            __import__("bass_rust").ScopedClock(
                {None: tick_clock.global_clock}
            ),
        )
        si = drain_inst.ins.sync_info
        if si is not None:
            last_lane = getattr(nc, "_last_hw_dma_lane", None)
            eng_w, dma_w = [], []
            for w in si.on_wait:
                name = getattr(w, "ant_name", "") or ""
                (dma_w if name.startswith("DMAHW") else eng_w).append(w)

            def lane_key(w):
                name = getattr(w, "ant_name", "") or ""
                try:
                    lane = int(name[5:].split("_")[0])
                except ValueError:
                    return 0
                if last_lane is None:
                    return lane
                return (lane - last_lane - 1) % 8

            dma_w.sort(key=lane_key)
            drain_inst.ins.sync_info = mybir.SyncInfo(
                on_wait=eng_w + dma_w, on_update=list(si.on_update)
            )
        nc.all_engine_barrier()
        assert self.sems is not None
        popped = nc._tile_sem_poison_stack.pop()
        assert popped is self._sem_poison
        nc.clear_and_free_semaphores(list(self.sems.allocated().values()))

    tile.TileContext._drain_and_barrier = _drain_and_barrier
    tile.TileContext._tail_single_barrier = True


def _cfg():
    def ilist(env, default):
        return [int(s) for s in os.environ.get(env, default).split(",")]

    return {
        "warmup": int(os.environ.get("KERNEL_WARMUP", "12")),
        # interleaved x/e chunk groups: (kind, n_chunks) pairs; x groups issue
        # on the SP ring and e groups on the Act ring, so both rings push
        # issues in parallel and the wire interleaves them by entry order
        "xe_stream": os.environ.get(
            "KERNEL_XE_STREAM", "x4,e4,x4,e4,x4,e4,x4,e4"
        ).split(","),
        "ht_groups": ilist("KERNEL_HT_GROUPS", "1,1,1,1"),
        # xeT column-slices (units of 256 n-columns, 8 units total)
        "xet_groups": ilist("KERNEL_XET_GROUPS", "1,2,2,1,1,1"),
        # the e-group index whose last pass2 matmul gates the ht/xeT issues
        # (keeps their wire-queue entries behind the whole e stream)
        "ht_gate": int(os.environ.get("KERNEL_HT_GATE", "-1")),
        # drain groups (n-chunks per PSUM accumulation group)
        "st_groups": ilist("KERNEL_ST_GROUPS", "2,2,2,2,1,1,1,1,1,1,1,1"),
        # how many consecutive drain groups share one store DMA
        "st_dma": ilist("KERNEL_ST_DMA", "2,2,2,2,2,2"),
        # ring per store DMA: y=sync(SP) a=scalar(Act) d=vector(DVE) p=pool
        "st_rings": os.environ.get("KERNEL_ST_RINGS", "y,y,y,y,a,y").split(","),
        "sym": os.environ.get("KERNEL_SYM", "1") == "1",
        "g_drain": os.environ.get("KERNEL_GDRAIN", "s,v,s,v").split(","),
        "m_drain": os.environ.get("KERNEL_MDRAIN", "v,s,v,s").split(","),
        "x_ring": os.environ.get("KERNEL_X_RING", "sync"),
        "e_ring": os.environ.get("KERNEL_E_RING", "scalar"),
        "in_ring": os.environ.get("KERNEL_IN_RING", "sync"),
        "last_drain": os.environ.get("KERNEL_LAST_DRAIN", "v"),
        "m_early": int(os.environ.get("KERNEL_M_EARLY", "0")),
        "g0_split": os.environ.get("KERNEL_G0_SPLIT", "0") == "1",
        "m0_split": os.environ.get("KERNEL_M0_SPLIT", "0") == "1",
        "g0_rot": int(os.environ.get("KERNEL_G0_ROT", "0")),
        "m_order": ilist("KERNEL_M_ORDER", "0,1,2,3"),
        "st_drain_par": int(os.environ.get("KERNEL_ST_DRAIN_PAR", "0")),
        "spread_pre": int(os.environ.get("KERNEL_SPREAD_PRE", "3")),
        "defer_bcreg": os.environ.get("KERNEL_DEFER_BCREG", "0") == "1",
        "tp_drain": os.environ.get("KERNEL_TP_DRAIN", "v"),
        "m_early_at": int(os.environ.get("KERNEL_M_EARLY_AT", "2")),
    }


def _build(cfg=None):
    import concourse.bass as bass
    import concourse.mybir as mybir
    import concourse.tile as tile
    from concourse import masks

    _patch_tail_barrier()

    if cfg is None:
        cfg = _cfg()

    f32 = mybir.dt.float32
    bf16 = mybir.dt.bfloat16

    nc = bass.Bass("TRN2", target_bir_lowering=False, debug=False)
    if cfg["defer_bcreg"]:
        # The preamble's 4 bounds-check register inits per engine (bcreg*,
        # all-ones = disabled) only need to precede that engine's first DMA,
        # not the start barrier.  Re-splice them to just after each engine's
        # barrier EventSemaphore: every engine then arrives at the barrier
        # ~200-380ns earlier and the whole kernel shifts with it.
        bb0 = nc.m.functions[0].blocks[0]
        insts = list(bb0.instructions)
        bcregs = [
            i
            for i in insts
            if type(i).__name__ == "InstRegisterMove"
            and any("bcreg" in str(o) for o in i.outs)
        ]
        rest = [i for i in insts if i not in bcregs]
        out_list = []
        for i in rest:
            out_list.append(i)
            if type(i).__name__ == "InstEventSemaphore":
                eng = i.engine
                for b in bcregs:
                    if b.engine == eng:
                        out_list.append(b)
                bcregs = [b for b in bcregs if b.engine != eng]
        out_list.extend(bcregs)
        bb0.instructions = out_list
    if cfg["spread_pre"]:
        # The Bass-init const-AP memsets all land on Pool, making Pool the
        # slowest arrival at the TileContext start barrier (~930ns vs ~550
        # for the next engine) -- the whole kernel hangs off that barrier.
        # Spread them across DVE/Act so every engine arrives by ~650ns.
        pre_ms = [
            i
            for bb in nc.m.functions[0].blocks
            for i in bb.instructions
            if type(i).__name__ == "InstMemset"
        ]
        for k, inst in enumerate(pre_ms):
            if k < cfg["spread_pre"]:
                inst.engine = mybir.EngineType.DVE
    xe_d = nc.dram_tensor("xe", (N, D), bf16, kind="ExternalInput").ap()
    xet_d = nc.dram_tensor("xeT", (D, N), bf16, kind="ExternalInput").ap()
    ht_d = nc.dram_tensor("HT", (D, D), bf16, kind="ExternalInput").ap()
    out_d = nc.dram_tensor("out", (N, T), bf16, kind="ExternalOutput").ap()

    # parse the interleaved xe stream (x/e chunk groups; "h" tokens place HT
    # j-chunk groups inline in the wire order; UPPERCASE X/E issue the group
    # on the OTHER ring, so e.g. "e2,E2" puts both e-tail halves in flight
    # concurrently on both rings)
    xe_stream = []  # (kind, chunk_slice, swap_ring)
    xpos = epos = hpos = 0
    for tokstr in cfg["xe_stream"]:
        kind, cnt = tokstr[0], int(tokstr[1:])
        swap = kind.isupper()
        kind = kind.lower()
        if kind == "x":
            xe_stream.append(("x", slice(xpos, xpos + cnt), swap))
            xpos += cnt
        elif kind == "h":
            xe_stream.append(("h", slice(hpos, hpos + cnt), swap))
            hpos += cnt
        else:
            xe_stream.append(("e", slice(epos, epos + cnt), swap))
            epos += cnt
    assert xpos == NCH and epos == NCH
    assert hpos + sum(cfg["ht_groups"]) == DCH
    assert sum(cfg["xet_groups"]) == NCH // 2
    st_groups = cfg["st_groups"]
    assert sum(st_groups) == NCH
    n_st = len(st_groups)
    st_dma = cfg["st_dma"]
    assert sum(st_dma) == n_st
    st_rings = cfg["st_rings"]
    assert len(st_rings) == len(st_dma)

    with tile.TileContext(nc) as tc:
        with (
            tc.tile_pool(name="consts", bufs=1) as consts,
            tc.tile_pool(name="outp", bufs=max(len(st_dma), 1)) as outp,
            tc.tile_pool(name="ps", bufs=8, space="PSUM") as ps,
        ):
            xe_sb = consts.tile([P, NCH, D], bf16)
            xet_sb = consts.tile([P, DCH, N], bf16)
            ht_sb = consts.tile([P, DCH, D], bf16)
            g_sb = consts.tile([P, DCH, T], bf16)
            m_sb = consts.tile([P, DCH, T], bf16)

            if cfg["warmup"]:
                wt = consts.tile([P, 64], f32)
                nc.gpsimd.memset(wt[:], 1.0)
            if cfg["sym"]:
                ident = consts.tile([P, P], bf16)
                masks.make_identity(nc, ident[:])

            # ---- PE p-state warmup: junk f32 matmuls (4 cycles/row) keep the
            # PE busy through the DMA spin-up window so the 3us half-speed
            # ramp is spent before real work arrives. ----
            if cfg["warmup"]:
                wp = ps.tile([P, 64], f32, tag="ps", name="warm")
                for i in range(cfg["warmup"]):
                    nc.tensor.matmul(
                        wp[0:64, :], wt[:, 0:64], wt[:], start=True, stop=True
                    )

            # ---- input DMA streams.  x groups issue on the SP ring and e
            # groups on the Act ring so both rings push issues concurrently
            # (one ring's ~650ns per-issue cost can't pace the fine-grained
            # interleave); the wire serves them in queue-entry order, which
            # matches the x/e interleave.  The ht/xeT stream (back on SP)
            # is GATED behind a PE-matmul semaphore so its wire-queue entries
            # stay behind the whole e stream instead of jumping ahead of the
            # e tail. ----
            xer = xe_d.rearrange("(c p) d -> p c d", p=P)
            xetr = xet_d.rearrange("(dc p) n -> p dc n", p=P)
            htr = ht_d.rearrange("(c p) j -> p c j", p=P)

            x_ring = getattr(nc, cfg["x_ring"])
            e_ring = getattr(nc, cfg["e_ring"])
            ring = getattr(nc, cfg["in_ring"])
            ht_done = 0
            for kind, arg, swap in xe_stream:
                if kind == "x":
                    r = e_ring if swap else x_ring
                    r.dma_start(xe_sb[:, arg, 0:T], xer[:, arg, 0:T])
                elif kind == "h":
                    ring.dma_start(ht_sb[:, arg, :], htr[:, arg, :])
                    ht_done = arg.stop
                else:
                    r = x_ring if swap else e_ring
                    r.dma_start(xe_sb[:, arg, T:D], xer[:, arg, T:D])
            gate_sem = nc.alloc_semaphore("htgate")
            first_ht_name = None
            c0 = ht_done
            for gsz in cfg["ht_groups"]:
                di = ring.dma_start(
                    ht_sb[:, c0 : c0 + gsz, :], htr[:, c0 : c0 + gsz, :]
                )
                if first_ht_name is None:
                    first_ht_name = di.ins.name
                c0 += gsz
            c0 = 0
            for gsz in cfg["xet_groups"]:
                n0, n1 = c0 * 2 * P, (c0 + gsz) * 2 * P
                ring.dma_start(xet_sb[:, :, n0:n1], xetr[:, :, n0:n1])
                c0 += gsz

            _cp = {
                "v": nc.vector.tensor_copy,
                "s": nc.scalar.copy,
                "p": nc.gpsimd.tensor_copy,
            }
            g_drain = cfg["g_drain"]

            # ---- G[j, t] = sum_n xe[n, j] x[n, t], pass1 (dc0/dc1, x rows)
            # and pass2 (dc2/dc3, e rows) interleaved per x/e chunk group in
            # wire arrival order.  With sym=1 pass1's dc1 row computes only
            # t in [128,256); the missing (1,0) tile is a PE transpose of the
            # drained (0,1) tile.  start=True clears has_written for the
            # WHOLE PSUM bank, so the two accumulators sharing a bank act as
            # one: start on the bank's first matmul, stop on its last. ----
            g_pair = [
                ps.tile([P, 2, T], f32, tag="ps", name=f"g_pair{i}")
                for i in range(DCH // 2)
            ]
            g_ps = [g_pair[dc // 2][:, dc % 2, :] for dc in range(DCH)]

            m_drain = cfg["m_drain"]
            mp = [ps.tile([P, T], f32, tag="ps", name=f"mp{jp}") for jp in range(DCH)]
            m_waves_done = 0

            def emit_m_waves(j_hi):
                # emit M accumulation waves in cfg["m_order"] (the PSUM
                # accumulation is j-order-free): running the transpose-
                # dependent wave (j=1) last hides the tp drain chain behind
                # the other waves
                nonlocal m_waves_done
                order = cfg["m_order"]
                for oi in range(m_waves_done, j_hi):
                    j = order[oi]
                    for jp in range(DCH):
                        nc.tensor.matmul(
                            mp[jp][:],
                            ht_sb[:, j, jp * P : (jp + 1) * P],
                            g_sb[:, j, :],
                            start=(oi == 0),
                            stop=(oi == DCH - 1),
                        )
                        if oi == DCH - 1:
                            if jp == 0 and cfg["m0_split"]:
                                # halve the out phase's gating latency: m0
                                # drains as two parallel halves on both
                                # copy engines
                                nc.vector.tensor_copy(
                                    m_sb[:, 0, 0 : T // 2], mp[0][:, 0 : T // 2]
                                )
                                nc.scalar.copy(
                                    m_sb[:, 0, T // 2 : T], mp[0][:, T // 2 : T]
                                )
                            else:
                                _cp[m_drain[jp]](m_sb[:, jp, :], mp[jp][:])
                m_waves_done = j_hi

            x_seen = e_seen = 0
            eg_idx = -1
            tp_emitted = False
            for kind, arg, _swap in xe_stream:
                chunks = range(arg.start, arg.stop)
                if kind == "h":
                    continue
                if kind == "x":
                    for c in chunks:
                        nc.tensor.matmul(
                            g_ps[0],
                            xe_sb[:, c, 0:P],
                            xe_sb[:, c, 0:T],
                            start=(c == 0),
                            stop=False,
                            skip_group_check=True,
                        )
                        if cfg["sym"]:
                            nc.tensor.matmul(
                                g_ps[1][:, P:T],
                                xe_sb[:, c, P : 2 * P],
                                xe_sb[:, c, P:T],
                                start=False,
                                stop=(c == NCH - 1),
                                skip_group_check=True,
                            )
                        else:
                            nc.tensor.matmul(
                                g_ps[1],
                                xe_sb[:, c, P : 2 * P],
                                xe_sb[:, c, 0:T],
                                start=False,
                                stop=(c == NCH - 1),
                                skip_group_check=True,
                            )
                    x_seen = arg.stop
                    if x_seen == NCH:
                        # pass1 closed: drain dc0/dc1 now so the symmetry
                        # transpose (emitted a few matmuls later) finds its
                        # input settled.  The [128:256] half of dc0 (the
                        # transpose's input) drains first on its own engine
                        # so the transpose chain doesn't wait the full row.
                        if cfg["g0_split"]:
                            _cp[g_drain[0]](g_sb[:, 0, P:T], g_ps[0][:, P:T])
                            opp = "v" if g_drain[0] == "s" else "s"
                            _cp[opp](g_sb[:, 0, 0:P], g_ps[0][:, 0:P])
                        else:
                            _cp[g_drain[0]](g_sb[:, 0, :], g_ps[0])
                        if cfg["sym"]:
                            _cp[g_drain[1]](g_sb[:, 1, P:T], g_ps[1][:, P:T])
                        else:
                            _cp[g_drain[1]](g_sb[:, 1, :], g_ps[1])
                else:
                    eg_idx += 1
                    for c in chunks:
                        for dc in (2, 3):
                            mm = nc.tensor.matmul(
                                g_ps[dc],
                                xe_sb[:, c, dc * P : (dc + 1) * P],
                                xe_sb[:, c, 0:T],
                                start=(c == 0 and dc == 2),
                                stop=(c == NCH - 1 and dc == 3),
                                skip_group_check=True,
                            )
                            if (
                                eg_idx == cfg["ht_gate"]
                                and c == chunks[-1]
                                and dc == 3
                            ):
                                mm.then_inc(gate_sem, 1)
                    e_seen = arg.stop
                if cfg["sym"] and not tp_emitted and x_seen == NCH and e_seen >= 2:
                    tp_ps = ps.tile([P, P], bf16, tag="ps", name="tp")
                    nc.tensor.transpose(tp_ps[:], g_sb[:, 0, P:T], ident[:])
                    _cp[cfg["tp_drain"]](g_sb[:, 1, 0:P], tp_ps[:])
                    tp_emitted = True
                # early M waves: j0/j1 need only pass1's G rows (drained once
                # x_seen==NCH) and the inline-loaded ht chunks, so they can
                # run between pass2 chunk groups instead of after all of G
                if (
                    cfg["m_early"] > m_waves_done
                    and x_seen == NCH
                    and (tp_emitted or not cfg["sym"])
                    and eg_idx >= cfg["m_early_at"]
                ):
                    emit_m_waves(min(cfg["m_early"], 2))
            if cfg["sym"] and not tp_emitted:
                tp_ps = ps.tile([P, P], bf16, tag="ps", name="tp")
                nc.tensor.transpose(tp_ps[:], g_sb[:, 0, P:T], ident[:])
                _cp[cfg["tp_drain"]](g_sb[:, 1, 0:P], tp_ps[:])
            _cp[g_drain[2]](g_sb[:, 2, :], g_ps[2])
            _cp[g_drain[3]](g_sb[:, 3, :], g_ps[3])

            # ---- M[j', t] = sum_j HT[j, j'] G[j, t]; one PSUM bank per
            # j'-chunk, j-outer so each wave consumes ht chunk j as it lands;
            # the last wave is interleaved with drains so m_sb[0] is ready
            # several matmuls before the wave ends ----
            emit_m_waves(DCH)

            # ---- out[n, t] = sum_j' xe[n, j'] M[j', t]; drain groups sized
            # by st_groups, several drain groups staged into one store DMA
            # (st_dma) on the ring given by st_rings.  The final group is one
            # chunk with its drain split across DVE+Act so the kernel tail
            # carries only half a drain + one store. ----
            ring_map = {
                "y": nc.sync,
                "a": nc.scalar,
                "d": nc.vector,
                "p": nc.gpsimd,
            }
            # store DMA si covers drain groups [st_lo[si], st_hi[si])
            st_lo, st_hi = [], []
            g0 = 0
            for cnt in st_dma:
                st_lo.append(g0)
                st_hi.append(g0 + cnt)
                g0 += cnt
            # staging tile per store DMA
            stg_tiles = []
            for si in range(len(st_dma)):
                nch = sum(st_groups[st_lo[si] : st_hi[si]])
                stg_tiles.append(
                    outp.tile([P, nch, T], bf16, tag=f"stg{si}", name=f"stg{si}")
                )
            # group -> (store idx, chunk offset within staging tile)
            g2s = {}
            for si in range(len(st_dma)):
                off = 0
                for gi in range(st_lo[si], st_hi[si]):
                    g2s[gi] = (si, off)
                    off += st_groups[gi]

            c0 = 0
            for gi, gsz in enumerate(st_groups):
                if gi == n_st - 1 and gsz == 1 and cfg["last_drain"] == "tsplit":
                    # final chunk: column-split into two PSUM tiles so the
                    # first half drains (258ns) while the second half's
                    # matmuls still run -- the store then waits only half a
                    # drain past the kernel's last matmul
                    si, off = g2s[gi]
                    stg = stg_tiles[si]
                    opA = ps.tile([P, T // 2], f32, tag="ps", name="opA")
                    opB = ps.tile([P, T // 2], f32, tag="ps", name="opB")
                    for half, oph in ((0, opA), (1, opB)):
                        t0 = half * (T // 2)
                        for dc in range(DCH):
                            nc.tensor.matmul(
                                oph[:],
                                xet_sb[:, dc, c0 * P : (c0 + 1) * P],
                                m_sb[:, dc, t0 : t0 + T // 2],
                                start=(dc == 0),
                                stop=(dc == DCH - 1),
                                skip_group_check=True,
                            )
                        nc.vector.tensor_copy(
                            stg[:, off : off + 1, t0 : t0 + T // 2], oph[:]
                        )
                    c0 += gsz
                    if gi == st_hi[si] - 1:
                        nch = sum(st_groups[st_lo[si] : st_hi[si]])
                        dst0 = c0 - nch
                        ring_map[st_rings[si]].dma_start(
                            out_d[dst0 * P : c0 * P, :].rearrange(
                                "(c p) t -> p c t", p=P
                            ),
                            stg[:],
                        )
                    continue
                op = ps.tile([P, gsz, T], f32, tag="ps", name=f"op{gi}")
                order = [(k, dc) for k in range(gsz) for dc in range(DCH)]
                if gi == 0:
                    # skew the first group dc-major so its dc3 matmul comes as
                    # late as possible: m_sb[3]'s drain is still in flight
                    # when the out phase reaches the head of the PE queue.
                    # g0_rot starts the dc sequence at whichever m-chunk
                    # drains earliest.
                    r = cfg["g0_rot"]
                    order.sort(key=lambda t: ((t[1] - r) % DCH, t[0]))
                last = order[-1]
                for k, dc in order:
                    nc.tensor.matmul(
                        op[:, k, :],
                        xet_sb[:, dc, (c0 + k) * P : (c0 + k + 1) * P],
                        m_sb[:, dc, :],
                        start=((k, dc) == order[0]),
                        stop=((k, dc) == last),
                        skip_group_check=True,
                    )
                si, off = g2s[gi]
                stg = stg_tiles[si]
                if gi == n_st - 1 and gsz == 1 and cfg["last_drain"] == "split":
                    # split the final drain across both copy engines
                    nc.vector.tensor_copy(
                        stg[:, off : off + gsz, 0 : T // 2], op[:, :, 0 : T // 2]
                    )
                    nc.scalar.copy(
                        stg[:, off : off + gsz, T // 2 : T], op[:, :, T // 2 : T]
                    )
                elif gi == n_st - 1 and cfg["last_drain"] in ("v", "s"):
                    _cp[cfg["last_drain"]](stg[:, off : off + gsz, :], op[:])
                else:
                    par = (gi + cfg["st_drain_par"]) % 2
                    eng = nc.vector.tensor_copy if par == 0 else nc.scalar.copy
                    eng(stg[:, off : off + gsz, :], op[:])
                c0 += gsz
                if gi == st_hi[si] - 1:
                    # last drain group of this store DMA: fire it
                    nch = stg.shape()[1] if callable(getattr(stg, "shape", None)) else sum(
                        st_groups[st_lo[si] : st_hi[si]]
                    )
                    nch = sum(st_groups[st_lo[si] : st_hi[si]])
                    dst0 = c0 - nch
                    ring_map[st_rings[si]].dma_start(
                        out_d[dst0 * P : c0 * P, :].rearrange(
                            "(c p) t -> p c t", p=P
                        ),
                        stg[:],
                    )

    # attach the gate wait to the first ht DMA (post-build so Tile's sem
    # assignment can't drop it): the ht/xeT stream may not enter the shared
    # wire queue before the designated pass2 matmul has executed
    n_e_groups = sum(1 for k, _, _ in xe_stream if k == "e")
    if first_ht_name is not None and 0 <= cfg["ht_gate"] < n_e_groups:
        for f in nc.m.functions:
            for bb in f.blocks:
                for inst in bb.instructions:
                    if inst.name == first_ht_name:
                        si = inst.sync_info
                        waits = list(si.on_wait) if si is not None else []
                        upds = list(si.on_update) if si is not None else []
                        waits.append(
                            mybir.SyncWait(
                                sync_type="semaphore",
                                id=gate_sem.num,
                                ant_name=gate_sem.name,
                                wait_mode="sem-ge-imm",
                                wait_value=1,
                            )
                        )
                        inst.sync_info = mybir.SyncInfo(
                            on_wait=waits, on_update=upds
                        )

    # record the DMAHW lane of the final store for the tail wait reordering
    n_hw_dma = 0
    for f in nc.m.functions:
        for bb in f.blocks:
            for inst in bb.instructions:
                if type(inst).__name__ in ("InstDMACopy", "InstDmaTransposeAnt"):
                    if str(inst.engine) != "EngineType.Pool":
                        n_hw_dma += 1
    nc._last_hw_dma_lane = (n_hw_dma - 1) % 8 if n_hw_dma else None

    _split_excess_waits(nc)
    return nc


def _get_nc():
    if "nc" not in _CACHE:
        _CACHE["nc"] = _build()
    return _CACHE["nc"]


def _prep_in_maps(inputs):
    import ml_dtypes

    bf = ml_dtypes.bfloat16
    x = np.asarray(inputs["x"], dtype=np.float32)
    e = np.asarray(inputs["e"], dtype=np.float32)
    wq = np.asarray(inputs["Wq"], dtype=np.float32)
    wk = np.asarray(inputs["Wk"], dtype=np.float32)

    ht = (SCALE * (wk.T @ wq)).astype(bf)  # H^T = SCALE * Wk^T Wq
    in_maps = []
    for b in range(B):
        xe = np.concatenate([x[b], e], axis=1).astype(bf)  # (N, D)
        xet = np.ascontiguousarray(xe.T)  # (D, N)
        in_maps.append({"xe": xe, "xeT": xet, "HT": ht})
    return in_maps


def _run(inputs, **kwargs):
    from concourse.bass_utils import run_bass_kernel_spmd

    in_maps = _prep_in_maps(inputs)
    res = run_bass_kernel_spmd(_get_nc(), in_maps, core_ids=list(range(B)), **kwargs)
    out = np.stack([np.asarray(r["out"]) for r in res.results], axis=0).astype(
        np.float32, copy=False
    )
    return out, res


def kernel(**inputs) -> np.ndarray:
    out, _ = _run(inputs)
    return out


# revision 68
# speedup vs baseline: 1.0018x; 1.0018x over previous
"""Bass/Tile kernel for nn_Causal_Temporal_Map_Attention_2 on 8 TRN2 NeuronCores.

Math: the reference is bilinear attention WITHOUT softmax:
    xe  = concat([x_b, e], -1)                    # (n, 512) per batch
    out = (xe Wq^T) (xe Wk^T)^T x_b * SCALE       # (n, 256)

By associativity this collapses to
    G   = xe^T x_b                                # (512, 256)   O(n d^2)
    M   = (SCALE * Wq^T Wk) G = H G               # (512, 256)
    out = xe M                                    # (n, 256)

Sharding is data-parallel over batch: core i handles batch element i
(b == n_cores == 8).

Device-side schedule (all matmuls bf16 with f32 PSUM accumulation):
  * warmup junk matmuls burn the PE p-state half-speed ramp during the DMA
    spin-up window.
  * one HWDGE input stream on the SP ring with x-half and e-half chunk
    groups INTERLEAVED, so the G phase's pass1 (x^T x rows) and pass2
    (e^T x rows) interleave per chunk group and G closes right behind the
    wire instead of serializing pass2 after the whole x stream.
  * G pass1 exploits the symmetry of the x^T x block: its (1,0) 128x128
    tile is a PE transpose of the (0,1) tile instead of 16 more matmuls.
  * out phase: PSUM chunk groups drain f32->bf16 into per-store staging
    tiles on alternating DVE/Act engines; several drain groups share one
    store DMA to keep the HWDGE slot count down.  The final group is a
    single chunk whose drain is split across both engines, so the kernel
    tail carries only half a drain + one store.
  * the kernel-end drain's semaphore waits are reordered so the final
    store's DMA lane is waited last: the one-wait-per-instruction NoOp
    chain then retires while that store's 900ns sem propagation is still
    in flight instead of after it.
"""

import os
import sys

if "/opt/trn_rl_repo" not in sys.path:
    sys.path.insert(0, "/opt/trn_rl_repo")

import numpy as np

B = 8
N = 2048
T = 256  # DIM_X
D = 512  # DIM_X + DIM_E
P = 128
NCH = N // P  # 16 sequence chunks
DCH = D // P  # 4 feature chunks
SCALE = float(D) ** -0.5

_CACHE = {}


def _split_excess_waits(nc, max_waits=1):
    """The walrus build in this container rejects instructions carrying more
    than one embedded semaphore wait ("Too many sync wait commands").  Tile's
    add_semaphores freely attaches 3+ (and the kernel-tail drain collects one
    per outstanding sem).  Rehome the excess onto nofuse NOPs prepended on the
    same engine -- the sequencer executes them in order, so blocking semantics
    are identical."""
    import concourse.mybir as mybir

    n_split = 0
    for f in nc.m.functions:
        for bb in f.blocks:
            new_insts = []
            for inst in bb.instructions:
                si = inst.sync_info
                waits = list(si.on_wait) if si is not None else []
                if len(waits) > max_waits:
                    excess = waits[: -max_waits]
                    keep = waits[-max_waits:]
                    for k in range(0, len(excess), max_waits):
                        chunk = excess[k : k + max_waits]
                        nop = mybir.InstNoOp(
                            name=f"{inst.name}-wsplit{k}",
                            engine=inst.engine,
                            ins=[],
                            outs=[],
                            text_hint="waitsplit",
                            bass_nofuse=True,
                            sync_info=mybir.SyncInfo(on_wait=chunk, on_update=[]),
                        )
                        new_insts.append(nop)
                        n_split += 1
                    inst.sync_info = mybir.SyncInfo(
                        on_wait=keep, on_update=list(si.on_update)
                    )
                new_insts.append(inst)
            bb.instructions = new_insts
    return n_split


def _patch_tail_barrier():
    """The stock kernel epilogue is drain -> all-engine barrier -> sem clear
    -> all-engine barrier.  The second barrier only keeps already-drained
    engines from halting before the sem clears land, which is harmless: NEFF
    completion requires every engine to halt, and the clearing engine halts
    after its clears.  Eliding it saves ~0.9us of tail.

    Additionally the drain's waits are reordered so the DMA-lane sems
    (DMAHW*) come last, the lane belonging to the final store very last:
    _split_excess_waits turns each wait into its own 50ns NoOp, and this
    ordering lets the early (engine) NoOps retire while the final store's
    DMA-sem propagation is still in flight."""
    import concourse.tile as tile

    if getattr(tile.TileContext, "_tail_single_barrier", False):
        return

    def _drain_and_barrier(self, tick_clock, wait_clock):
        import concourse.mybir as mybir

        nc = self.nc
        drain_inst = nc.sync.drain()
        wait_clock.add_sem_waits(
            drain_inst.ins,
            __import__("bass_rust").ScopedClock(
                {None: tick_clock.global_clock}
            ),
        )
        si = drain_inst.ins.sync_info
        if si is not None:
            last_lane = getattr(nc, "_last_hw_dma_lane", None)
            eng_w, dma_w = [], []
            for w in si.on_wait:
                name = getattr(w, "ant_name", "") or ""
                (dma_w if name.startswith("DMAHW") else eng_w).append(w)

            def lane_key(w):
                name = getattr(w, "ant_name", "") or ""
                try:
                    lane = int(name[5:].split("_")[0])
                except ValueError:
                    return 0
                if last_lane is None:
                    return lane
                return (lane - last_lane - 1) % 8

            dma_w.sort(key=lane_key)
            drain_inst.ins.sync_info = mybir.SyncInfo(
                on_wait=eng_w + dma_w, on_update=list(si.on_update)
            )
        nc.all_engine_barrier()
        assert self.sems is not None
        popped = nc._tile_sem_poison_stack.pop()
        assert popped is self._sem_poison
        nc.clear_and_free_semaphores(list(self.sems.allocated().values()))

    tile.TileContext._drain_and_barrier = _drain_and_barrier
    tile.TileContext._tail_single_barrier = True


def _cfg():
    def ilist(env, default):
        return [int(s) for s in os.environ.get(env, default).split(",")]

    return {
        "warmup": int(os.environ.get("KERNEL_WARMUP", "12")),
        # interleaved x/e chunk groups: (kind, n_chunks) pairs; x groups issue
        # on the SP ring and e groups on the Act ring, so both rings push
        # issues in parallel and the wire interleaves them by entry order
        "xe_stream": os.environ.get(
            "KERNEL_XE_STREAM", "x4,e4,x4,e4,x4,e4,x4,e4"
        ).split(","),
        "ht_groups": ilist("KERNEL_HT_GROUPS", "1,1,1,1"),
        # xeT column-slices (units of 256 n-columns, 8 units total)
        "xet_groups": ilist("KERNEL_XET_GROUPS", "1,2,2,1,1,1"),
        # the e-group index whose last pass2 matmul gates the ht/xeT issues
        # (keeps their wire-queue entries behind the whole e stream)
        "ht_gate": int(os.environ.get("KERNEL_HT_GATE", "-1")),
        # drain groups (n-chunks per PSUM accumulation group)
        "st_groups": ilist("KERNEL_ST_GROUPS", "2,2,2,2,1,1,1,1,1,1,1,1"),
        # how many consecutive drain groups share one store DMA
        "st_dma": ilist("KERNEL_ST_DMA", "2,2,2,2,2,2"),
        # ring per store DMA: y=sync(SP) a=scalar(Act) d=vector(DVE) p=pool
        "st_rings": os.environ.get("KERNEL_ST_RINGS", "y,y,y,y,a,y").split(","),
        "sym": os.environ.get("KERNEL_SYM", "1") == "1",
        "g_drain": os.environ.get("KERNEL_GDRAIN", "s,v,s,v").split(","),
        "m_drain": os.environ.get("KERNEL_MDRAIN", "v,s,v,s").split(","),
        "x_ring": os.environ.get("KERNEL_X_RING", "sync"),
        "e_ring": os.environ.get("KERNEL_E_RING", "scalar"),
        "in_ring": os.environ.get("KERNEL_IN_RING", "sync"),
        "last_drain": os.environ.get("KERNEL_LAST_DRAIN", "v"),
        "m_early": int(os.environ.get("KERNEL_M_EARLY", "0")),
        "g0_split": os.environ.get("KERNEL_G0_SPLIT", "0") == "1",
        "m0_split": os.environ.get("KERNEL_M0_SPLIT", "0") == "1",
        "g0_rot": int(os.environ.get("KERNEL_G0_ROT", "0")),
        "m_order": ilist("KERNEL_M_ORDER", "0,1,2,3"),
        "st_drain_par": int(os.environ.get("KERNEL_ST_DRAIN_PAR", "0")),
        "spread_pre": int(os.environ.get("KERNEL_SPREAD_PRE", "3")),
        "defer_bcreg": os.environ.get("KERNEL_DEFER_BCREG", "pe+dve"),
        "tp_drain": os.environ.get("KERNEL_TP_DRAIN", "v"),
        "m_early_at": int(os.environ.get("KERNEL_M_EARLY_AT", "2")),
    }


def _build(cfg=None):
    import concourse.bass as bass
    import concourse.mybir as mybir
    import concourse.tile as tile
    from concourse import masks

    _patch_tail_barrier()

    if cfg is None:
        cfg = _cfg()

    f32 = mybir.dt.float32
    bf16 = mybir.dt.bfloat16

    nc = bass.Bass("TRN2", target_bir_lowering=False, debug=False)
    if cfg["defer_bcreg"]:
        defer_set = set(cfg["defer_bcreg"].split("+"))
        # The preamble's 4 bounds-check register inits per engine (bcreg*,
        # all-ones = disabled) only need to precede that engine's first DMA,
        # not the start barrier.  Re-splice them to just after each engine's
        # barrier EventSemaphore: every engine then arrives at the barrier
        # ~200-380ns earlier and the whole kernel shifts with it.
        bb0 = nc.m.functions[0].blocks[0]
        insts = list(bb0.instructions)
        eng_names = {"pe": "PE", "dve": "DVE", "pool": "Pool", "sp": "SP", "act": "Activation"}
        targets = {eng_names[e] for e in defer_set if e in eng_names}
        bcregs = [
            i
            for i in insts
            if type(i).__name__ == "InstRegisterMove"
            and any("bcreg" in str(o) for o in i.outs)
            and str(i.engine).split(".")[-1] in targets
        ]
        rest = [i for i in insts if i not in bcregs]
        out_list = []
        for i in rest:
            out_list.append(i)
            if type(i).__name__ == "InstEventSemaphore":
                eng = i.engine
                for b in bcregs:
                    if b.engine == eng:
                        out_list.append(b)
                bcregs = [b for b in bcregs if b.engine != eng]
        out_list.extend(bcregs)
        bb0.instructions = out_list
    if cfg["spread_pre"]:
        # The Bass-init const-AP memsets all land on Pool, making Pool the
        # slowest arrival at the TileContext start barrier (~930ns vs ~550
        # for the next engine) -- the whole kernel hangs off that barrier.
        # Spread them across DVE/Act so every engine arrives by ~650ns.
        pre_ms = [
            i
            for bb in nc.m.functions[0].blocks
            for i in bb.instructions
            if type(i).__name__ == "InstMemset"
        ]
        for k, inst in enumerate(pre_ms):
            if k < cfg["spread_pre"]:
                inst.engine = mybir.EngineType.DVE
    xe_d = nc.dram_tensor("xe", (N, D), bf16, kind="ExternalInput").ap()
    xet_d = nc.dram_tensor("xeT", (D, N), bf16, kind="ExternalInput").ap()
    ht_d = nc.dram_tensor("HT", (D, D), bf16, kind="ExternalInput").ap()
    out_d = nc.dram_tensor("out", (N, T), bf16, kind="ExternalOutput").ap()

    # parse the interleaved xe stream (x/e chunk groups; "h" tokens place HT
    # j-chunk groups inline in the wire order; UPPERCASE X/E issue the group
    # on the OTHER ring, so e.g. "e2,E2" puts both e-tail halves in flight
    # concurrently on both rings)
    xe_stream = []  # (kind, chunk_slice, swap_ring)
    xpos = epos = hpos = 0
    for tokstr in cfg["xe_stream"]:
        kind, cnt = tokstr[0], int(tokstr[1:])
        swap = kind.isupper()
        kind = kind.lower()
        if kind == "x":
            xe_stream.append(("x", slice(xpos, xpos + cnt), swap))
            xpos += cnt
        elif kind == "h":
            xe_stream.append(("h", slice(hpos, hpos + cnt), swap))
            hpos += cnt
        else:
            xe_stream.append(("e", slice(epos, epos + cnt), swap))
            epos += cnt
    assert xpos == NCH and epos == NCH
    assert hpos + sum(cfg["ht_groups"]) == DCH
    assert sum(cfg["xet_groups"]) == NCH // 2
    st_groups = cfg["st_groups"]
    assert sum(st_groups) == NCH
    n_st = len(st_groups)
    st_dma = cfg["st_dma"]
    assert sum(st_dma) == n_st
    st_rings = cfg["st_rings"]
    assert len(st_rings) == len(st_dma)

    with tile.TileContext(nc) as tc:
        with (
            tc.tile_pool(name="consts", bufs=1) as consts,
            tc.tile_pool(name="outp", bufs=max(len(st_dma), 1)) as outp,
            tc.tile_pool(name="ps", bufs=8, space="PSUM") as ps,
        ):
            xe_sb = consts.tile([P, NCH, D], bf16)
            xet_sb = consts.tile([P, DCH, N], bf16)
            ht_sb = consts.tile([P, DCH, D], bf16)
            g_sb = consts.tile([P, DCH, T], bf16)
            m_sb = consts.tile([P, DCH, T], bf16)

            if cfg["warmup"]:
                wt = consts.tile([P, 64], f32)
                nc.gpsimd.memset(wt[:], 1.0)
            if cfg["sym"]:
                ident = consts.tile([P, P], bf16)
                masks.make_identity(nc, ident[:])

            # ---- PE p-state warmup: junk f32 matmuls (4 cycles/row) keep the
            # PE busy through the DMA spin-up window so the 3us half-speed
            # ramp is spent before real work arrives. ----
            if cfg["warmup"]:
                wp = ps.tile([P, 64], f32, tag="ps", name="warm")
                for i in range(cfg["warmup"]):
                    nc.tensor.matmul(
                        wp[0:64, :], wt[:, 0:64], wt[:], start=True, stop=True
                    )

            # ---- input DMA streams.  x groups issue on the SP ring and e
            # groups on the Act ring so both rings push issues concurrently
            # (one ring's ~650ns per-issue cost can't pace the fine-grained
            # interleave); the wire serves them in queue-entry order, which
            # matches the x/e interleave.  The ht/xeT stream (back on SP)
            # is GATED behind a PE-matmul semaphore so its wire-queue entries
            # stay behind the whole e stream instead of jumping ahead of the
            # e tail. ----
            xer = xe_d.rearrange("(c p) d -> p c d", p=P)
            xetr = xet_d.rearrange("(dc p) n -> p dc n", p=P)
            htr = ht_d.rearrange("(c p) j -> p c j", p=P)

            x_ring = getattr(nc, cfg["x_ring"])
            e_ring = getattr(nc, cfg["e_ring"])
            ring = getattr(nc, cfg["in_ring"])
            ht_done = 0
            for kind, arg, swap in xe_stream:
                if kind == "x":
                    r = e_ring if swap else x_ring
                    r.dma_start(xe_sb[:, arg, 0:T], xer[:, arg, 0:T])
                elif kind == "h":
                    ring.dma_start(ht_sb[:, arg, :], htr[:, arg, :])
                    ht_done = arg.stop
                else:
                    r = x_ring if swap else e_ring
                    r.dma_start(xe_sb[:, arg, T:D], xer[:, arg, T:D])
            gate_sem = nc.alloc_semaphore("htgate")
            first_ht_name = None
            c0 = ht_done
            for gsz in cfg["ht_groups"]:
                di = ring.dma_start(
                    ht_sb[:, c0 : c0 + gsz, :], htr[:, c0 : c0 + gsz, :]
                )
                if first_ht_name is None:
                    first_ht_name = di.ins.name
                c0 += gsz
            c0 = 0
            for gsz in cfg["xet_groups"]:
                n0, n1 = c0 * 2 * P, (c0 + gsz) * 2 * P
                ring.dma_start(xet_sb[:, :, n0:n1], xetr[:, :, n0:n1])
                c0 += gsz

            _cp = {
                "v": nc.vector.tensor_copy,
                "s": nc.scalar.copy,
                "p": nc.gpsimd.tensor_copy,
            }
            g_drain = cfg["g_drain"]

            # ---- G[j, t] = sum_n xe[n, j] x[n, t], pass1 (dc0/dc1, x rows)
            # and pass2 (dc2/dc3, e rows) interleaved per x/e chunk group in
            # wire arrival order.  With sym=1 pass1's dc1 row computes only
            # t in [128,256); the missing (1,0) tile is a PE transpose of the
            # drained (0,1) tile.  start=True clears has_written for the
            # WHOLE PSUM bank, so the two accumulators sharing a bank act as
            # one: start on the bank's first matmul, stop on its last. ----
            g_pair = [
                ps.tile([P, 2, T], f32, tag="ps", name=f"g_pair{i}")
                for i in range(DCH // 2)
            ]
            g_ps = [g_pair[dc // 2][:, dc % 2, :] for dc in range(DCH)]

            m_drain = cfg["m_drain"]
            mp = [ps.tile([P, T], f32, tag="ps", name=f"mp{jp}") for jp in range(DCH)]
            m_waves_done = 0

            def emit_m_waves(j_hi):
                # emit M accumulation waves in cfg["m_order"] (the PSUM
                # accumulation is j-order-free): running the transpose-
                # dependent wave (j=1) last hides the tp drain chain behind
                # the other waves
                nonlocal m_waves_done
                order = cfg["m_order"]
                for oi in range(m_waves_done, j_hi):
                    j = order[oi]
                    for jp in range(DCH):
                        nc.tensor.matmul(
                            mp[jp][:],
                            ht_sb[:, j, jp * P : (jp + 1) * P],
                            g_sb[:, j, :],
                            start=(oi == 0),
                            stop=(oi == DCH - 1),
                        )
                        if oi == DCH - 1:
                            if jp == 0 and cfg["m0_split"]:
                                # halve the out phase's gating latency: m0
                                # drains as two parallel halves on both
                                # copy engines
                                nc.vector.tensor_copy(
                                    m_sb[:, 0, 0 : T // 2], mp[0][:, 0 : T // 2]
                                )
                                nc.scalar.copy(
                                    m_sb[:, 0, T // 2 : T], mp[0][:, T // 2 : T]
                                )
                            else:
                                _cp[m_drain[jp]](m_sb[:, jp, :], mp[jp][:])
                m_waves_done = j_hi

            x_seen = e_seen = 0
            eg_idx = -1
            tp_emitted = False
            for kind, arg, _swap in xe_stream:
                chunks = range(arg.start, arg.stop)
                if kind == "h":
                    continue
                if kind == "x":
                    for c in chunks:
                        nc.tensor.matmul(
                            g_ps[0],
                            xe_sb[:, c, 0:P],
                            xe_sb[:, c, 0:T],
                            start=(c == 0),
                            stop=False,
                            skip_group_check=True,
                        )
                        if cfg["sym"]:
                            nc.tensor.matmul(
                                g_ps[1][:, P:T],
                                xe_sb[:, c, P : 2 * P],
                                xe_sb[:, c, P:T],
                                start=False,
                                stop=(c == NCH - 1),
                                skip_group_check=True,
                            )
                        else:
                            nc.tensor.matmul(
                                g_ps[1],
                                xe_sb[:, c, P : 2 * P],
                                xe_sb[:, c, 0:T],
                                start=False,
                                stop=(c == NCH - 1),
                                skip_group_check=True,
                            )
                    x_seen = arg.stop
                    if x_seen == NCH:
                        # pass1 closed: drain dc0/dc1 now so the symmetry
                        # transpose (emitted a few matmuls later) finds its
                        # input settled.  The [128:256] half of dc0 (the
                        # transpose's input) drains first on its own engine
                        # so the transpose chain doesn't wait the full row.
                        if cfg["g0_split"]:
                            _cp[g_drain[0]](g_sb[:, 0, P:T], g_ps[0][:, P:T])
                            opp = "v" if g_drain[0] == "s" else "s"
                            _cp[opp](g_sb[:, 0, 0:P], g_ps[0][:, 0:P])
                        else:
                            _cp[g_drain[0]](g_sb[:, 0, :], g_ps[0])
                        if cfg["sym"]:
                            _cp[g_drain[1]](g_sb[:, 1, P:T], g_ps[1][:, P:T])
                        else:
                            _cp[g_drain[1]](g_sb[:, 1, :], g_ps[1])
                else:
                    eg_idx += 1
                    for c in chunks:
                        for dc in (2, 3):
                            mm = nc.tensor.matmul(
                                g_ps[dc],
                                xe_sb[:, c, dc * P : (dc + 1) * P],
                                xe_sb[:, c, 0:T],
                                start=(c == 0 and dc == 2),
                                stop=(c == NCH - 1 and dc == 3),
                                skip_group_check=True,
                            )
                            if (
                                eg_idx == cfg["ht_gate"]
                                and c == chunks[-1]
                                and dc == 3
                            ):
                                mm.then_inc(gate_sem, 1)
                    e_seen = arg.stop
                if cfg["sym"] and not tp_emitted and x_seen == NCH and e_seen >= 2:
                    tp_ps = ps.tile([P, P], bf16, tag="ps", name="tp")
                    nc.tensor.transpose(tp_ps[:], g_sb[:, 0, P:T], ident[:])
                    _cp[cfg["tp_drain"]](g_sb[:, 1, 0:P], tp_ps[:])
                    tp_emitted = True
                # early M waves: j0/j1 need only pass1's G rows (drained once
                # x_seen==NCH) and the inline-loaded ht chunks, so they can
                # run between pass2 chunk groups instead of after all of G
                if (
                    cfg["m_early"] > m_waves_done
                    and x_seen == NCH
                    and (tp_emitted or not cfg["sym"])
                    and eg_idx >= cfg["m_early_at"]
                ):
                    emit_m_waves(min(cfg["m_early"], 2))
            if cfg["sym"] and not tp_emitted:
                tp_ps = ps.tile([P, P], bf16, tag="ps", name="tp")
                nc.tensor.transpose(tp_ps[:], g_sb[:, 0, P:T], ident[:])
                _cp[cfg["tp_drain"]](g_sb[:, 1, 0:P], tp_ps[:])
            _cp[g_drain[2]](g_sb[:, 2, :], g_ps[2])
            _cp[g_drain[3]](g_sb[:, 3, :], g_ps[3])

            # ---- M[j', t] = sum_j HT[j, j'] G[j, t]; one PSUM bank per
            # j'-chunk, j-outer so each wave consumes ht chunk j as it lands;
            # the last wave is interleaved with drains so m_sb[0] is ready
            # several matmuls before the wave ends ----
            emit_m_waves(DCH)

            # ---- out[n, t] = sum_j' xe[n, j'] M[j', t]; drain groups sized
            # by st_groups, several drain groups staged into one store DMA
            # (st_dma) on the ring given by st_rings.  The final group is one
            # chunk with its drain split across DVE+Act so the kernel tail
            # carries only half a drain + one store. ----
            ring_map = {
                "y": nc.sync,
                "a": nc.scalar,
                "d": nc.vector,
                "p": nc.gpsimd,
            }
            # store DMA si covers drain groups [st_lo[si], st_hi[si])
            st_lo, st_hi = [], []
            g0 = 0
            for cnt in st_dma:
                st_lo.append(g0)
                st_hi.append(g0 + cnt)
                g0 += cnt
            # staging tile per store DMA
            stg_tiles = []
            for si in range(len(st_dma)):
                nch = sum(st_groups[st_lo[si] : st_hi[si]])
                stg_tiles.append(
                    outp.tile([P, nch, T], bf16, tag=f"stg{si}", name=f"stg{si}")
                )
            # group -> (store idx, chunk offset within staging tile)
            g2s = {}
            for si in range(len(st_dma)):
                off = 0
                for gi in range(st_lo[si], st_hi[si]):
                    g2s[gi] = (si, off)
                    off += st_groups[gi]

            c0 = 0
            for gi, gsz in enumerate(st_groups):
                if gi == n_st - 1 and gsz == 1 and cfg["last_drain"] == "tsplit":
                    # final chunk: column-split into two PSUM tiles so the
                    # first half drains (258ns) while the second half's
                    # matmuls still run -- the store then waits only half a
                    # drain past the kernel's last matmul
                    si, off = g2s[gi]
                    stg = stg_tiles[si]
                    opA = ps.tile([P, T // 2], f32, tag="ps", name="opA")
                    opB = ps.tile([P, T // 2], f32, tag="ps", name="opB")
                    for half, oph in ((0, opA), (1, opB)):
                        t0 = half * (T // 2)
                        for dc in range(DCH):
                            nc.tensor.matmul(
                                oph[:],
                                xet_sb[:, dc, c0 * P : (c0 + 1) * P],
                                m_sb[:, dc, t0 : t0 + T // 2],
                                start=(dc == 0),
                                stop=(dc == DCH - 1),
                                skip_group_check=True,
                            )
                        nc.vector.tensor_copy(
                            stg[:, off : off + 1, t0 : t0 + T // 2], oph[:]
                        )
                    c0 += gsz
                    if gi == st_hi[si] - 1:
                        nch = sum(st_groups[st_lo[si] : st_hi[si]])
                        dst0 = c0 - nch
                        ring_map[st_rings[si]].dma_start(
                            out_d[dst0 * P : c0 * P, :].rearrange(
                                "(c p) t -> p c t", p=P
                            ),
                            stg[:],
                        )
                    continue
                op = ps.tile([P, gsz, T], f32, tag="ps", name=f"op{gi}")
                order = [(k, dc) for k in range(gsz) for dc in range(DCH)]
                if gi == 0:
                    # skew the first group dc-major so its dc3 matmul comes as
                    # late as possible: m_sb[3]'s drain is still in flight
                    # when the out phase reaches the head of the PE queue.
                    # g0_rot starts the dc sequence at whichever m-chunk
                    # drains earliest.
                    r = cfg["g0_rot"]
                    order.sort(key=lambda t: ((t[1] - r) % DCH, t[0]))
                last = order[-1]
                for k, dc in order:
                    nc.tensor.matmul(
                        op[:, k, :],
                        xet_sb[:, dc, (c0 + k) * P : (c0 + k + 1) * P],
                        m_sb[:, dc, :],
                        start=((k, dc) == order[0]),
                        stop=((k, dc) == last),
                        skip_group_check=True,
                    )
                si, off = g2s[gi]
                stg = stg_tiles[si]
                if gi == n_st - 1 and gsz == 1 and cfg["last_drain"] == "split":
                    # split the final drain across both copy engines
                    nc.vector.tensor_copy(
                        stg[:, off : off + gsz, 0 : T // 2], op[:, :, 0 : T // 2]
                    )
                    nc.scalar.copy(
                        stg[:, off : off + gsz, T // 2 : T], op[:, :, T // 2 : T]
                    )
                elif gi == n_st - 1 and cfg["last_drain"] in ("v", "s"):
                    _cp[cfg["last_drain"]](stg[:, off : off + gsz, :], op[:])
                else:
                    par = (gi + cfg["st_drain_par"]) % 2
                    eng = nc.vector.tensor_copy if par == 0 else nc.scalar.copy
                    eng(stg[:, off : off + gsz, :], op[:])
                c0 += gsz
                if gi == st_hi[si] - 1:
                    # last drain group of this store DMA: fire it
                    nch = stg.shape()[1] if callable(getattr(stg, "shape", None)) else sum(
                        st_groups[st_lo[si] : st_hi[si]]
                    )
                    nch = sum(st_groups[st_lo[si] : st_hi[si]])
                    dst0 = c0 - nch
                    ring_map[st_rings[si]].dma_start(
                        out_d[dst0 * P : c0 * P, :].rearrange(
                            "(c p) t -> p c t", p=P
                        ),
                        stg[:],
                    )

    # attach the gate wait to the first ht DMA (post-build so Tile's sem
    # assignment can't drop it): the ht/xeT stream may not enter the shared
    # wire queue before the designated pass2 matmul has executed
    n_e_groups = sum(1 for k, _, _ in xe_stream if k == "e")
    if first_ht_name is not None and 0 <= cfg["ht_gate"] < n_e_groups:
        for f in nc.m.functions:
            for bb in f.blocks:
                for inst in bb.instructions:
                    if inst.name == first_ht_name:
                        si = inst.sync_info
                        waits = list(si.on_wait) if si is not None else []
                        upds = list(si.on_update) if si is not None else []
                        waits.append(
                            mybir.SyncWait(
                                sync_type="semaphore",
                                id=gate_sem.num,
                                ant_name=gate_sem.name,
                                wait_mode="sem-ge-imm",
                                wait_value=1,
                            )
                        )
                        inst.sync_info = mybir.SyncInfo(
                            on_wait=waits, on_update=upds
                        )

    # record the DMAHW lane of the final store for the tail wait reordering
    n_hw_dma = 0
    for f in nc.m.functions:
        for bb in f.blocks:
            for inst in bb.instructions:
                if type(inst).__name__ in ("InstDMACopy", "InstDmaTransposeAnt"):
                    if str(inst.engine) != "EngineType.Pool":
                        n_hw_dma += 1
    nc._last_hw_dma_lane = (n_hw_dma - 1) % 8 if n_hw_dma else None

    _split_excess_waits(nc)
    return nc


def _get_nc():
    if "nc" not in _CACHE:
        _CACHE["nc"] = _build()
    return _CACHE["nc"]


def _prep_in_maps(inputs):
    import ml_dtypes

    bf = ml_dtypes.bfloat16
    x = np.asarray(inputs["x"], dtype=np.float32)
    e = np.asarray(inputs["e"], dtype=np.float32)
    wq = np.asarray(inputs["Wq"], dtype=np.float32)
    wk = np.asarray(inputs["Wk"], dtype=np.float32)

    ht = (SCALE * (wk.T @ wq)).astype(bf)  # H^T = SCALE * Wk^T Wq
    in_maps = []
    for b in range(B):
        xe = np.concatenate([x[b], e], axis=1).astype(bf)  # (N, D)
        xet = np.ascontiguousarray(xe.T)  # (D, N)
        in_maps.append({"xe": xe, "xeT": xet, "HT": ht})
    return in_maps


def _run(inputs, **kwargs):
    from concourse.bass_utils import run_bass_kernel_spmd

    in_maps = _prep_in_maps(inputs)
    res = run_bass_kernel_spmd(_get_nc(), in_maps, core_ids=list(range(B)), **kwargs)
    out = np.stack([np.asarray(r["out"]) for r in res.results], axis=0).astype(
        np.float32, copy=False
    )
    return out, res


def kernel(**inputs) -> np.ndarray:
    out, _ = _run(inputs)
    return out


# revision 69
# speedup vs baseline: 1.0071x; 1.0054x over previous
"""Bass/Tile kernel for nn_Causal_Temporal_Map_Attention_2 on 8 TRN2 NeuronCores.

Math: the reference is bilinear attention WITHOUT softmax:
    xe  = concat([x_b, e], -1)                    # (n, 512) per batch
    out = (xe Wq^T) (xe Wk^T)^T x_b * SCALE       # (n, 256)

By associativity this collapses to
    G   = xe^T x_b                                # (512, 256)   O(n d^2)
    M   = (SCALE * Wq^T Wk) G = H G               # (512, 256)
    out = xe M                                    # (n, 256)

Sharding is data-parallel over batch: core i handles batch element i
(b == n_cores == 8).

Device-side schedule (all matmuls bf16 with f32 PSUM accumulation):
  * warmup junk matmuls burn the PE p-state half-speed ramp during the DMA
    spin-up window.
  * one HWDGE input stream on the SP ring with x-half and e-half chunk
    groups INTERLEAVED, so the G phase's pass1 (x^T x rows) and pass2
    (e^T x rows) interleave per chunk group and G closes right behind the
    wire instead of serializing pass2 after the whole x stream.
  * G pass1 exploits the symmetry of the x^T x block: its (1,0) 128x128
    tile is a PE transpose of the (0,1) tile instead of 16 more matmuls.
  * out phase: PSUM chunk groups drain f32->bf16 into per-store staging
    tiles on alternating DVE/Act engines; several drain groups share one
    store DMA to keep the HWDGE slot count down.  The final group is a
    single chunk whose drain is split across both engines, so the kernel
    tail carries only half a drain + one store.
  * the kernel-end drain's semaphore waits are reordered so the final
    store's DMA lane is waited last: the one-wait-per-instruction NoOp
    chain then retires while that store's 900ns sem propagation is still
    in flight instead of after it.
"""

import os
import sys

if "/opt/trn_rl_repo" not in sys.path:
    sys.path.insert(0, "/opt/trn_rl_repo")

import numpy as np

B = 8
N = 2048
T = 256  # DIM_X
D = 512  # DIM_X + DIM_E
P = 128
NCH = N // P  # 16 sequence chunks
DCH = D // P  # 4 feature chunks
SCALE = float(D) ** -0.5

_CACHE = {}


def _split_excess_waits(nc, max_waits=1):
    """The walrus build in this container rejects instructions carrying more
    than one embedded semaphore wait ("Too many sync wait commands").  Tile's
    add_semaphores freely attaches 3+ (and the kernel-tail drain collects one
    per outstanding sem).  Rehome the excess onto nofuse NOPs prepended on the
    same engine -- the sequencer executes them in order, so blocking semantics
    are identical."""
    import concourse.mybir as mybir

    n_split = 0
    for f in nc.m.functions:
        for bb in f.blocks:
            new_insts = []
            for inst in bb.instructions:
                si = inst.sync_info
                waits = list(si.on_wait) if si is not None else []
                if len(waits) > max_waits:
                    excess = waits[: -max_waits]
                    keep = waits[-max_waits:]
                    for k in range(0, len(excess), max_waits):
                        chunk = excess[k : k + max_waits]
                        nop = mybir.InstNoOp(
                            name=f"{inst.name}-wsplit{k}",
                            engine=inst.engine,
                            ins=[],
                            outs=[],
                            text_hint="waitsplit",
                            bass_nofuse=True,
                            sync_info=mybir.SyncInfo(on_wait=chunk, on_update=[]),
                        )
                        new_insts.append(nop)
                        n_split += 1
                    inst.sync_info = mybir.SyncInfo(
                        on_wait=keep, on_update=list(si.on_update)
                    )
                new_insts.append(inst)
            bb.instructions = new_insts
    return n_split


def _patch_tail_barrier():
    """The stock kernel epilogue is drain -> all-engine barrier -> sem clear
    -> all-engine barrier.  The second barrier only keeps already-drained
    engines from halting before the sem clears land, which is harmless: NEFF
    completion requires every engine to halt, and the clearing engine halts
    after its clears.  Eliding it saves ~0.9us of tail.

    Additionally the drain's waits are reordered so the DMA-lane sems
    (DMAHW*) come last, the lane belonging to the final store very last:
    _split_excess_waits turns each wait into its own 50ns NoOp, and this
    ordering lets the early (engine) NoOps retire while the final store's
    DMA-sem propagation is still in flight."""
    import concourse.tile as tile

    if getattr(tile.TileContext, "_tail_single_barrier", False):
        return

    def _drain_and_barrier(self, tick_clock, wait_clock):
        import concourse.mybir as mybir

        nc = self.nc
        drain_inst = nc.sync.drain()
        wait_clock.add_sem_waits(
            drain_inst.ins,
            __import__("bass_rust").ScopedClock(
                {None: tick_clock.global_clock}
            ),
        )
        si = drain_inst.ins.sync_info
        if si is not None:
            last_lane = getattr(nc, "_last_hw_dma_lane", None)
            eng_w, dma_w = [], []
            for w in si.on_wait:
                name = getattr(w, "ant_name", "") or ""
                (dma_w if name.startswith("DMAHW") else eng_w).append(w)

            def lane_key(w):
                name = getattr(w, "ant_name", "") or ""
                try:
                    lane = int(name[5:].split("_")[0])
                except ValueError:
                    return 0
                if last_lane is None:
                    return lane
                return (lane - last_lane - 1) % 8

            dma_w.sort(key=lane_key)
            drain_inst.ins.sync_info = mybir.SyncInfo(
                on_wait=eng_w + dma_w, on_update=list(si.on_update)
            )
        nc.all_engine_barrier()
        assert self.sems is not None
        popped = nc._tile_sem_poison_stack.pop()
        assert popped is self._sem_poison
        nc.clear_and_free_semaphores(list(self.sems.allocated().values()))

    tile.TileContext._drain_and_barrier = _drain_and_barrier
    tile.TileContext._tail_single_barrier = True


def _cfg():
    def ilist(env, default):
        return [int(s) for s in os.environ.get(env, default).split(",")]

    return {
        "warmup": int(os.environ.get("KERNEL_WARMUP", "12")),
        # interleaved x/e chunk groups: (kind, n_chunks) pairs; x groups issue
        # on the SP ring and e groups on the Act ring, so both rings push
        # issues in parallel and the wire interleaves them by entry order
        "xe_stream": os.environ.get(
            "KERNEL_XE_STREAM", "x4,e4,x4,e4,x4,e4,x4,e4"
        ).split(","),
        "ht_groups": ilist("KERNEL_HT_GROUPS", "1,1,1,1"),
        # xeT column-slices (units of 256 n-columns, 8 units total)
        "xet_groups": ilist("KERNEL_XET_GROUPS", "1,2,2,1,1,1"),
        # the e-group index whose last pass2 matmul gates the ht/xeT issues
        # (keeps their wire-queue entries behind the whole e stream)
        "ht_gate": int(os.environ.get("KERNEL_HT_GATE", "-1")),
        # drain groups (n-chunks per PSUM accumulation group)
        "st_groups": ilist("KERNEL_ST_GROUPS", "2,2,2,2,1,1,1,1,1,1,1,1"),
        # how many consecutive drain groups share one store DMA
        "st_dma": ilist("KERNEL_ST_DMA", "2,2,2,2,2,2"),
        # ring per store DMA: y=sync(SP) a=scalar(Act) d=vector(DVE) p=pool
        "st_rings": os.environ.get("KERNEL_ST_RINGS", "y,y,y,y,a,y").split(","),
        "sym": os.environ.get("KERNEL_SYM", "1") == "1",
        "g_drain": os.environ.get("KERNEL_GDRAIN", "s,v,s,v").split(","),
        "m_drain": os.environ.get("KERNEL_MDRAIN", "v,s,v,s").split(","),
        "x_ring": os.environ.get("KERNEL_X_RING", "sync"),
        "e_ring": os.environ.get("KERNEL_E_RING", "scalar"),
        "in_ring": os.environ.get("KERNEL_IN_RING", "sync"),
        "last_drain": os.environ.get("KERNEL_LAST_DRAIN", "v"),
        "m_early": int(os.environ.get("KERNEL_M_EARLY", "0")),
        "g0_split": os.environ.get("KERNEL_G0_SPLIT", "0") == "1",
        "m0_split": os.environ.get("KERNEL_M0_SPLIT", "0") == "1",
        "g0_rot": int(os.environ.get("KERNEL_G0_ROT", "0")),
        "m_order": ilist("KERNEL_M_ORDER", "0,1,2,3"),
        "st_drain_par": int(os.environ.get("KERNEL_ST_DRAIN_PAR", "0")),
        "spread_pre": int(os.environ.get("KERNEL_SPREAD_PRE", "4")),
        "defer_bcreg": os.environ.get("KERNEL_DEFER_BCREG", "pe+dve"),
        "tp_drain": os.environ.get("KERNEL_TP_DRAIN", "v"),
        "m_early_at": int(os.environ.get("KERNEL_M_EARLY_AT", "2")),
    }


def _build(cfg=None):
    import concourse.bass as bass
    import concourse.mybir as mybir
    import concourse.tile as tile
    from concourse import masks

    _patch_tail_barrier()

    if cfg is None:
        cfg = _cfg()

    f32 = mybir.dt.float32
    bf16 = mybir.dt.bfloat16

    nc = bass.Bass("TRN2", target_bir_lowering=False, debug=False)
    if cfg["defer_bcreg"]:
        defer_set = set(cfg["defer_bcreg"].split("+"))
        # The preamble's 4 bounds-check register inits per engine (bcreg*,
        # all-ones = disabled) only need to precede that engine's first DMA,
        # not the start barrier.  Re-splice them to just after each engine's
        # barrier EventSemaphore: every engine then arrives at the barrier
        # ~200-380ns earlier and the whole kernel shifts with it.
        bb0 = nc.m.functions[0].blocks[0]
        insts = list(bb0.instructions)
        eng_names = {"pe": "PE", "dve": "DVE", "pool": "Pool", "sp": "SP", "act": "Activation"}
        targets = {eng_names[e] for e in defer_set if e in eng_names}
        bcregs = [
            i
            for i in insts
            if type(i).__name__ == "InstRegisterMove"
            and any("bcreg" in str(o) for o in i.outs)
            and str(i.engine).split(".")[-1] in targets
        ]
        rest = [i for i in insts if i not in bcregs]
        out_list = []
        for i in rest:
            out_list.append(i)
            if type(i).__name__ == "InstEventSemaphore":
                eng = i.engine
                for b in bcregs:
                    if b.engine == eng:
                        out_list.append(b)
                bcregs = [b for b in bcregs if b.engine != eng]
        out_list.extend(bcregs)
        bb0.instructions = out_list
    if cfg["spread_pre"]:
        # The Bass-init const-AP memsets all land on Pool, making Pool the
        # slowest arrival at the TileContext start barrier (~930ns vs ~550
        # for the next engine) -- the whole kernel hangs off that barrier.
        # Spread them across DVE/Act so every engine arrives by ~650ns.
        pre_ms = [
            i
            for bb in nc.m.functions[0].blocks
            for i in bb.instructions
            if type(i).__name__ == "InstMemset"
        ]
        for k, inst in enumerate(pre_ms):
            if k < cfg["spread_pre"]:
                inst.engine = mybir.EngineType.DVE
    xe_d = nc.dram_tensor("xe", (N, D), bf16, kind="ExternalInput").ap()
    xet_d = nc.dram_tensor("xeT", (D, N), bf16, kind="ExternalInput").ap()
    ht_d = nc.dram_tensor("HT", (D, D), bf16, kind="ExternalInput").ap()
    out_d = nc.dram_tensor("out", (N, T), bf16, kind="ExternalOutput").ap()

    # parse the interleaved xe stream (x/e chunk groups; "h" tokens place HT
    # j-chunk groups inline in the wire order; UPPERCASE X/E issue the group
    # on the OTHER ring, so e.g. "e2,E2" puts both e-tail halves in flight
    # concurrently on both rings)
    xe_stream = []  # (kind, chunk_slice, swap_ring)
    xpos = epos = hpos = 0
    for tokstr in cfg["xe_stream"]:
        kind, cnt = tokstr[0], int(tokstr[1:])
        swap = kind.isupper()
        kind = kind.lower()
        if kind == "x":
            xe_stream.append(("x", slice(xpos, xpos + cnt), swap))
            xpos += cnt
        elif kind == "h":
            xe_stream.append(("h", slice(hpos, hpos + cnt), swap))
            hpos += cnt
        else:
            xe_stream.append(("e", slice(epos, epos + cnt), swap))
            epos += cnt
    assert xpos == NCH and epos == NCH
    assert hpos + sum(cfg["ht_groups"]) == DCH
    assert sum(cfg["xet_groups"]) == NCH // 2
    st_groups = cfg["st_groups"]
    assert sum(st_groups) == NCH
    n_st = len(st_groups)
    st_dma = cfg["st_dma"]
    assert sum(st_dma) == n_st
    st_rings = cfg["st_rings"]
    assert len(st_rings) == len(st_dma)

    with tile.TileContext(nc) as tc:
        with (
            tc.tile_pool(name="consts", bufs=1) as consts,
            tc.tile_pool(name="outp", bufs=max(len(st_dma), 1)) as outp,
            tc.tile_pool(name="ps", bufs=8, space="PSUM") as ps,
        ):
            xe_sb = consts.tile([P, NCH, D], bf16)
            xet_sb = consts.tile([P, DCH, N], bf16)
            ht_sb = consts.tile([P, DCH, D], bf16)
            g_sb = consts.tile([P, DCH, T], bf16)
            m_sb = consts.tile([P, DCH, T], bf16)

            if cfg["warmup"]:
                wt = consts.tile([P, 64], f32)
                nc.gpsimd.memset(wt[:], 1.0)
            if cfg["sym"]:
                ident = consts.tile([P, P], bf16)
                masks.make_identity(nc, ident[:])

            # ---- PE p-state warmup: junk f32 matmuls (4 cycles/row) keep the
            # PE busy through the DMA spin-up window so the 3us half-speed
            # ramp is spent before real work arrives. ----
            if cfg["warmup"]:
                wp = ps.tile([P, 64], f32, tag="ps", name="warm")
                for i in range(cfg["warmup"]):
                    nc.tensor.matmul(
                        wp[0:64, :], wt[:, 0:64], wt[:], start=True, stop=True
                    )

            # ---- input DMA streams.  x groups issue on the SP ring and e
            # groups on the Act ring so both rings push issues concurrently
            # (one ring's ~650ns per-issue cost can't pace the fine-grained
            # interleave); the wire serves them in queue-entry order, which
            # matches the x/e interleave.  The ht/xeT stream (back on SP)
            # is GATED behind a PE-matmul semaphore so its wire-queue entries
            # stay behind the whole e stream instead of jumping ahead of the
            # e tail. ----
            xer = xe_d.rearrange("(c p) d -> p c d", p=P)
            xetr = xet_d.rearrange("(dc p) n -> p dc n", p=P)
            htr = ht_d.rearrange("(c p) j -> p c j", p=P)

            x_ring = getattr(nc, cfg["x_ring"])
            e_ring = getattr(nc, cfg["e_ring"])
            ring = getattr(nc, cfg["in_ring"])
            ht_done = 0
            for kind, arg, swap in xe_stream:
                if kind == "x":
                    r = e_ring if swap else x_ring
                    r.dma_start(xe_sb[:, arg, 0:T], xer[:, arg, 0:T])
                elif kind == "h":
                    ring.dma_start(ht_sb[:, arg, :], htr[:, arg, :])
                    ht_done = arg.stop
                else:
                    r = x_ring if swap else e_ring
                    r.dma_start(xe_sb[:, arg, T:D], xer[:, arg, T:D])
            gate_sem = nc.alloc_semaphore("htgate")
            first_ht_name = None
            c0 = ht_done
            for gsz in cfg["ht_groups"]:
                di = ring.dma_start(
                    ht_sb[:, c0 : c0 + gsz, :], htr[:, c0 : c0 + gsz, :]
                )
                if first_ht_name is None:
                    first_ht_name = di.ins.name
                c0 += gsz
            c0 = 0
            for gsz in cfg["xet_groups"]:
                n0, n1 = c0 * 2 * P, (c0 + gsz) * 2 * P
                ring.dma_start(xet_sb[:, :, n0:n1], xetr[:, :, n0:n1])
                c0 += gsz

            _cp = {
                "v": nc.vector.tensor_copy,
                "s": nc.scalar.copy,
                "p": nc.gpsimd.tensor_copy,
            }
            g_drain = cfg["g_drain"]

            # ---- G[j, t] = sum_n xe[n, j] x[n, t], pass1 (dc0/dc1, x rows)
            # and pass2 (dc2/dc3, e rows) interleaved per x/e chunk group in
            # wire arrival order.  With sym=1 pass1's dc1 row computes only
            # t in [128,256); the missing (1,0) tile is a PE transpose of the
            # drained (0,1) tile.  start=True clears has_written for the
            # WHOLE PSUM bank, so the two accumulators sharing a bank act as
            # one: start on the bank's first matmul, stop on its last. ----
            g_pair = [
                ps.tile([P, 2, T], f32, tag="ps", name=f"g_pair{i}")
                for i in range(DCH // 2)
            ]
            g_ps = [g_pair[dc // 2][:, dc % 2, :] for dc in range(DCH)]

            m_drain = cfg["m_drain"]
            mp = [ps.tile([P, T], f32, tag="ps", name=f"mp{jp}") for jp in range(DCH)]
            m_waves_done = 0

            def emit_m_waves(j_hi):
                # emit M accumulation waves in cfg["m_order"] (the PSUM
                # accumulation is j-order-free): running the transpose-
                # dependent wave (j=1) last hides the tp drain chain behind
                # the other waves
                nonlocal m_waves_done
                order = cfg["m_order"]
                for oi in range(m_waves_done, j_hi):
                    j = order[oi]
                    for jp in range(DCH):
                        nc.tensor.matmul(
                            mp[jp][:],
                            ht_sb[:, j, jp * P : (jp + 1) * P],
                            g_sb[:, j, :],
                            start=(oi == 0),
                            stop=(oi == DCH - 1),
                        )
                        if oi == DCH - 1:
                            if jp == 0 and cfg["m0_split"]:
                                # halve the out phase's gating latency: m0
                                # drains as two parallel halves on both
                                # copy engines
                                nc.vector.tensor_copy(
                                    m_sb[:, 0, 0 : T // 2], mp[0][:, 0 : T // 2]
                                )
                                nc.scalar.copy(
                                    m_sb[:, 0, T // 2 : T], mp[0][:, T // 2 : T]
                                )
                            else:
                                _cp[m_drain[jp]](m_sb[:, jp, :], mp[jp][:])
                m_waves_done = j_hi

            x_seen = e_seen = 0
            eg_idx = -1
            tp_emitted = False
            for kind, arg, _swap in xe_stream:
                chunks = range(arg.start, arg.stop)
                if kind == "h":
                    continue
                if kind == "x":
                    for c in chunks:
                        nc.tensor.matmul(
                            g_ps[0],
                            xe_sb[:, c, 0:P],
                            xe_sb[:, c, 0:T],
                            start=(c == 0),
                            stop=False,
                            skip_group_check=True,
                        )
                        if cfg["sym"]:
                            nc.tensor.matmul(
                                g_ps[1][:, P:T],
                                xe_sb[:, c, P : 2 * P],
                                xe_sb[:, c, P:T],
                                start=False,
                                stop=(c == NCH - 1),
                                skip_group_check=True,
                            )
                        else:
                            nc.tensor.matmul(
                                g_ps[1],
                                xe_sb[:, c, P : 2 * P],
                                xe_sb[:, c, 0:T],
                                start=False,
                                stop=(c == NCH - 1),
                                skip_group_check=True,
                            )
                    x_seen = arg.stop
                    if x_seen == NCH:
                        # pass1 closed: drain dc0/dc1 now so the symmetry
                        # transpose (emitted a few matmuls later) finds its
                        # input settled.  The [128:256] half of dc0 (the
                        # transpose's input) drains first on its own engine
                        # so the transpose chain doesn't wait the full row.
                        if cfg["g0_split"]:
                            _cp[g_drain[0]](g_sb[:, 0, P:T], g_ps[0][:, P:T])
                            opp = "v" if g_drain[0] == "s" else "s"
                            _cp[opp](g_sb[:, 0, 0:P], g_ps[0][:, 0:P])
                        else:
                            _cp[g_drain[0]](g_sb[:, 0, :], g_ps[0])
                        if cfg["sym"]:
                            _cp[g_drain[1]](g_sb[:, 1, P:T], g_ps[1][:, P:T])
                        else:
                            _cp[g_drain[1]](g_sb[:, 1, :], g_ps[1])
                else:
                    eg_idx += 1
                    for c in chunks:
                        for dc in (2, 3):
                            mm = nc.tensor.matmul(
                                g_ps[dc],
                                xe_sb[:, c, dc * P : (dc + 1) * P],
                                xe_sb[:, c, 0:T],
                                start=(c == 0 and dc == 2),
                                stop=(c == NCH - 1 and dc == 3),
                                skip_group_check=True,
                            )
                            if (
                                eg_idx == cfg["ht_gate"]
                                and c == chunks[-1]
                                and dc == 3
                            ):
                                mm.then_inc(gate_sem, 1)
                    e_seen = arg.stop
                if cfg["sym"] and not tp_emitted and x_seen == NCH and e_seen >= 2:
                    tp_ps = ps.tile([P, P], bf16, tag="ps", name="tp")
                    nc.tensor.transpose(tp_ps[:], g_sb[:, 0, P:T], ident[:])
                    _cp[cfg["tp_drain"]](g_sb[:, 1, 0:P], tp_ps[:])
                    tp_emitted = True
                # early M waves: j0/j1 need only pass1's G rows (drained once
                # x_seen==NCH) and the inline-loaded ht chunks, so they can
                # run between pass2 chunk groups instead of after all of G
                if (
                    cfg["m_early"] > m_waves_done
                    and x_seen == NCH
                    and (tp_emitted or not cfg["sym"])
                    and eg_idx >= cfg["m_early_at"]
                ):
                    emit_m_waves(min(cfg["m_early"], 2))
            if cfg["sym"] and not tp_emitted:
                tp_ps = ps.tile([P, P], bf16, tag="ps", name="tp")
                nc.tensor.transpose(tp_ps[:], g_sb[:, 0, P:T], ident[:])
                _cp[cfg["tp_drain"]](g_sb[:, 1, 0:P], tp_ps[:])
            _cp[g_drain[2]](g_sb[:, 2, :], g_ps[2])
            _cp[g_drain[3]](g_sb[:, 3, :], g_ps[3])

            # ---- M[j', t] = sum_j HT[j, j'] G[j, t]; one PSUM bank per
            # j'-chunk, j-outer so each wave consumes ht chunk j as it lands;
            # the last wave is interleaved with drains so m_sb[0] is ready
            # several matmuls before the wave ends ----
            emit_m_waves(DCH)

            # ---- out[n, t] = sum_j' xe[n, j'] M[j', t]; drain groups sized
            # by st_groups, several drain groups staged into one store DMA
            # (st_dma) on the ring given by st_rings.  The final group is one
            # chunk with its drain split across DVE+Act so the kernel tail
            # carries only half a drain + one store. ----
            ring_map = {
                "y": nc.sync,
                "a": nc.scalar,
                "d": nc.vector,
                "p": nc.gpsimd,
            }
            # store DMA si covers drain groups [st_lo[si], st_hi[si])
            st_lo, st_hi = [], []
            g0 = 0
            for cnt in st_dma:
                st_lo.append(g0)
                st_hi.append(g0 + cnt)
                g0 += cnt
            # staging tile per store DMA
            stg_tiles = []
            for si in range(len(st_dma)):
                nch = sum(st_groups[st_lo[si] : st_hi[si]])
                stg_tiles.append(
                    outp.tile([P, nch, T], bf16, tag=f"stg{si}", name=f"stg{si}")
                )
            # group -> (store idx, chunk offset within staging tile)
            g2s = {}
            for si in range(len(st_dma)):
                off = 0
                for gi in range(st_lo[si], st_hi[si]):
                    g2s[gi] = (si, off)
                    off += st_groups[gi]

            c0 = 0
            for gi, gsz in enumerate(st_groups):
                if gi == n_st - 1 and gsz == 1 and cfg["last_drain"] == "tsplit":
                    # final chunk: column-split into two PSUM tiles so the
                    # first half drains (258ns) while the second half's
                    # matmuls still run -- the store then waits only half a
                    # drain past the kernel's last matmul
                    si, off = g2s[gi]
                    stg = stg_tiles[si]
                    opA = ps.tile([P, T // 2], f32, tag="ps", name="opA")
                    opB = ps.tile([P, T // 2], f32, tag="ps", name="opB")
                    for half, oph in ((0, opA), (1, opB)):
                        t0 = half * (T // 2)
                        for dc in range(DCH):
                            nc.tensor.matmul(
                                oph[:],
                                xet_sb[:, dc, c0 * P : (c0 + 1) * P],
                                m_sb[:, dc, t0 : t0 + T // 2],
                                start=(dc == 0),
                                stop=(dc == DCH - 1),
                                skip_group_check=True,
                            )
                        nc.vector.tensor_copy(
                            stg[:, off : off + 1, t0 : t0 + T // 2], oph[:]
                        )
                    c0 += gsz
                    if gi == st_hi[si] - 1:
                        nch = sum(st_groups[st_lo[si] : st_hi[si]])
                        dst0 = c0 - nch
                        ring_map[st_rings[si]].dma_start(
                            out_d[dst0 * P : c0 * P, :].rearrange(
                                "(c p) t -> p c t", p=P
                            ),
                            stg[:],
                        )
                    continue
                op = ps.tile([P, gsz, T], f32, tag="ps", name=f"op{gi}")
                order = [(k, dc) for k in range(gsz) for dc in range(DCH)]
                if gi == 0:
                    # skew the first group dc-major so its dc3 matmul comes as
                    # late as possible: m_sb[3]'s drain is still in flight
                    # when the out phase reaches the head of the PE queue.
                    # g0_rot starts the dc sequence at whichever m-chunk
                    # drains earliest.
                    r = cfg["g0_rot"]
                    order.sort(key=lambda t: ((t[1] - r) % DCH, t[0]))
                last = order[-1]
                for k, dc in order:
                    nc.tensor.matmul(
                        op[:, k, :],
                        xet_sb[:, dc, (c0 + k) * P : (c0 + k + 1) * P],
                        m_sb[:, dc, :],
                        start=((k, dc) == order[0]),
                        stop=((k, dc) == last),
                        skip_group_check=True,
                    )
                si, off = g2s[gi]
                stg = stg_tiles[si]
                if gi == n_st - 1 and gsz == 1 and cfg["last_drain"] == "split":
                    # split the final drain across both copy engines
                    nc.vector.tensor_copy(
                        stg[:, off : off + gsz, 0 : T // 2], op[:, :, 0 : T // 2]
                    )
                    nc.scalar.copy(
                        stg[:, off : off + gsz, T // 2 : T], op[:, :, T // 2 : T]
                    )
                elif gi == n_st - 1 and cfg["last_drain"] in ("v", "s"):
                    _cp[cfg["last_drain"]](stg[:, off : off + gsz, :], op[:])
                else:
                    par = (gi + cfg["st_drain_par"]) % 2
                    eng = nc.vector.tensor_copy if par == 0 else nc.scalar.copy
                    eng(stg[:, off : off + gsz, :], op[:])
                c0 += gsz
                if gi == st_hi[si] - 1:
                    # last drain group of this store DMA: fire it
                    nch = stg.shape()[1] if callable(getattr(stg, "shape", None)) else sum(
                        st_groups[st_lo[si] : st_hi[si]]
                    )
                    nch = sum(st_groups[st_lo[si] : st_hi[si]])
                    dst0 = c0 - nch
                    ring_map[st_rings[si]].dma_start(
                        out_d[dst0 * P : c0 * P, :].rearrange(
                            "(c p) t -> p c t", p=P
                        ),
                        stg[:],
                    )

    # attach the gate wait to the first ht DMA (post-build so Tile's sem
    # assignment can't drop it): the ht/xeT stream may not enter the shared
    # wire queue before the designated pass2 matmul has executed
    n_e_groups = sum(1 for k, _, _ in xe_stream if k == "e")
    if first_ht_name is not None and 0 <= cfg["ht_gate"] < n_e_groups:
        for f in nc.m.functions:
            for bb in f.blocks:
                for inst in bb.instructions:
                    if inst.name == first_ht_name:
                        si = inst.sync_info
                        waits = list(si.on_wait) if si is not None else []
                        upds = list(si.on_update) if si is not None else []
                        waits.append(
                            mybir.SyncWait(
                                sync_type="semaphore",
                                id=gate_sem.num,
                                ant_name=gate_sem.name,
                                wait_mode="sem-ge-imm",
                                wait_value=1,
                            )
                        )
                        inst.sync_info = mybir.SyncInfo(
                            on_wait=waits, on_update=upds
                        )

    # record the DMAHW lane of the final store for the tail wait reordering
    n_hw_dma = 0
    for f in nc.m.functions:
        for bb in f.blocks:
            for inst in bb.instructions:
                if type(inst).__name__ in ("InstDMACopy", "InstDmaTransposeAnt"):
                    if str(inst.engine) != "EngineType.Pool":
                        n_hw_dma += 1
    nc._last_hw_dma_lane = (n_hw_dma - 1) % 8 if n_hw_dma else None

    _split_excess_waits(nc)
    return nc


def _get_nc():
    if "nc" not in _CACHE:
        _CACHE["nc"] = _build()
    return _CACHE["nc"]


def _prep_in_maps(inputs):
    import ml_dtypes

    bf = ml_dtypes.bfloat16
    x = np.asarray(inputs["x"], dtype=np.float32)
    e = np.asarray(inputs["e"], dtype=np.float32)
    wq = np.asarray(inputs["Wq"], dtype=np.float32)
    wk = np.asarray(inputs["Wk"], dtype=np.float32)

    ht = (SCALE * (wk.T @ wq)).astype(bf)  # H^T = SCALE * Wk^T Wq
    in_maps = []
    for b in range(B):
        xe = np.concatenate([x[b], e], axis=1).astype(bf)  # (N, D)
        xet = np.ascontiguousarray(xe.T)  # (D, N)
        in_maps.append({"xe": xe, "xeT": xet, "HT": ht})
    return in_maps


def _run(inputs, **kwargs):
    from concourse.bass_utils import run_bass_kernel_spmd

    in_maps = _prep_in_maps(inputs)
    res = run_bass_kernel_spmd(_get_nc(), in_maps, core_ids=list(range(B)), **kwargs)
    out = np.stack([np.asarray(r["out"]) for r in res.results], axis=0).astype(
        np.float32, copy=False
    )
    return out, res


def kernel(**inputs) -> np.ndarray:
    out, _ = _run(inputs)
    return out
